# revision 13
# baseline (speedup 1.0000x reference)
"""GATv2 backbone (3 layers, 4 heads x 64) on 8 Trainium2 NeuronCores.

Strategy (graph/data parallel):
  - dst-shard nodes: core k owns dst nodes [k*6250, (k+1)*6250), padded to 6272.
  - edges assigned to owner of dst, sorted by (dst-block of 128, src-half),
    padded per group to a size shared by all cores (SPMD: one program).
  - per layer: node-local xl/xr matmuls (fp16 out) -> AllGather xl ->
    edge phase: dma_gather rows of xl/xr per edge, DVE add+leaky+att-mul,
    per-head reduce -> logits -> exp (no max shift: logits in [-3, 3]) ->
    indicator-matrix matmul accumulates [msg | ex] into PSUM per dst block
    (segment softmax numerator + denominator in one pass) -> normalize,
    bias, relu, layernorm per block.
"""
import math
import numpy as np

# ---------------- problem constants (hardcoded per contract) ----------------
N_NODES = 50000
N_EDGES = 800000
IN_DIM = 128
HIDDEN = 64
HEADS = 4
HC = HEADS * HIDDEN  # 256
NUM_LAYERS = 3
NEG_SLOPE = 0.2
LN_EPS = 1e-5
NCORES = 8

_RUNNER = {}


def _cfg_default():
    return dict(n=N_NODES, e=N_EDGES, in_dim=IN_DIM, ncores=NCORES)


def _derive(cfg):
    n, ncores = cfg["n"], cfg["ncores"]
    dpc = n // ncores                      # dst nodes per core
    assert dpc * ncores == n
    dpcp = ((dpc + 127) // 128) * 128      # padded
    nfull = ncores * dpcp
    half = nfull // 2
    assert half % 128 == 0
    nb = dpcp // 128                       # dst blocks per core
    return dpc, dpcp, nfull, half, nb


def _wrap16(flat):
    """[S] int16 -> [128, S//16] wrapped-in-16-partitions, replicated x8."""
    assert flat.shape[0] % 16 == 0
    seg = flat.reshape(-1, 16).T  # [16, S/16]
    return np.tile(seg, (8, 1)).astype(np.int16)


def _preprocess(edge_index, cfg):
    """Build per-core edge structures with sizes shared across cores."""
    n, ncores = cfg["n"], cfg["ncores"]
    dpc, dpcp, nfull, half, nb = _derive(cfg)

    ei = np.asarray(edge_index).astype(np.int64)
    loop = np.arange(n, dtype=np.int64)
    src = np.concatenate([ei[0], loop])
    dst = np.concatenate([ei[1], loop])

    owner = dst // dpc
    dl = dst - owner * dpc                 # local dst 0..dpc-1
    blk = dl // 128
    gsrc = (src // dpc) * dpcp + (src % dpc)   # row in xl_full layout
    sh = (gsrc >= half).astype(np.int64)       # src half

    # group sizes per (core, block, half)
    key = (owner * nb + blk) * 2 + sh
    sizes = np.bincount(key, minlength=ncores * nb * 2).reshape(ncores, nb, 2)
    pad_sizes = sizes.max(axis=0)          # [nb, 2] shared across cores
    pad_sizes = ((pad_sizes + 127) // 128) * 128

    nsub_b = pad_sizes.sum(axis=1) // 128  # subtiles per block
    nsub_total = int(nsub_b.sum())
    e_pad = nsub_total * 128

    per_core = []
    for k in range(ncores):
        m = owner == k
        gs_k, dl_k, blk_k, sh_k = gsrc[m], dl[m], blk[m], sh[m]
        src_l = np.zeros(e_pad, np.int64)
        dst_l = np.zeros(e_pad, np.int64)
        real = np.zeros(e_pad, bool)
        pos = 0
        src_cols = []
        dst_cols = []
        for b in range(nb):
            bpos = pos
            for h in (0, 1):
                S = int(pad_sizes[b, h])
                if S == 0:
                    continue
                sel = (blk_k == b) & (sh_k == h)
                cnt = int(sel.sum())
                src_l[pos:pos + cnt] = gs_k[sel] - h * half
                dst_l[pos:pos + cnt] = dl_k[sel]
                real[pos:pos + cnt] = True
                # pad: src row 0 of this half, dst = b*128 (masked by Ind=0)
                dst_l[pos + cnt:pos + S] = b * 128
                src_cols.append(_wrap16(src_l[pos:pos + S].astype(np.int16)))
                pos += S
            Sb = pos - bpos
            if Sb:
                dst_cols.append(_wrap16(dst_l[bpos:pos].astype(np.int16)))
        assert pos == e_pad
        # indicator: [e_part 128, nsub, 128 dst] fp16
        st_of_edge = np.arange(e_pad) // 128
        blk_of_st = np.repeat(
            np.concatenate([np.full(int(nsub_b[b]), b) for b in range(nb)]), 128
        )
        d_in_blk = dst_l - blk_of_st * 128
        ind = np.zeros((e_pad, 128), np.float16)
        rr = np.nonzero(real)[0]
        ind[rr, d_in_blk[rr]] = 1.0
        ind_dev = np.ascontiguousarray(
            ind.reshape(nsub_total, 128, 128).transpose(1, 0, 2).reshape(128, -1)
        )
        per_core.append(
            dict(
                srcidx=np.ascontiguousarray(np.concatenate(src_cols, axis=1)),
                dstidx=np.ascontiguousarray(np.concatenate(dst_cols, axis=1)),
                ind=ind_dev,
            )
        )
    meta = dict(pad_sizes=pad_sizes, nsub_b=nsub_b, nsub_total=nsub_total, e_pad=e_pad)
    return per_core, meta


def _build_program(params_np, meta, cfg):
    import concourse.bass as bass
    import concourse.bacc as bacc
    import concourse.mybir as mybir
    import concourse.tile as tile

    dt = mybir.dt
    AF = mybir.ActivationFunctionType
    OP = mybir.AluOpType

    n, ncores, in_dim = cfg["n"], cfg["ncores"], cfg["in_dim"]
    dpc, dpcp, nfull, half, nb = _derive(cfg)
    pad_sizes, nsub_b = meta["pad_sizes"], meta["nsub_b"]
    nsub_total, e_pad = meta["nsub_total"], meta["e_pad"]

    ED = dt.float16  # edge-phase dtype

    n_layers = cfg.get("layers", NUM_LAYERS)
    max_blocks = cfg.get("max_blocks", 10**9)
    edge_ops = cfg.get("edge_ops", 9)
    import os
    skip_xlg = bool(os.environ.get("SKIP_XLG"))
    skip_xrg = bool(os.environ.get("SKIP_XRG"))
    skip_ind = bool(os.environ.get("SKIP_IND"))
    skip_node = bool(os.environ.get("SKIP_NODE"))
    nc = bacc.Bacc("TRN2", target_bir_lowering=False, debug=False,
                   num_devices=ncores)

    h0 = nc.dram_tensor("h0", [dpcp, in_dim], dt.float32, kind="ExternalInput")
    srcidx_d = nc.dram_tensor("srcidx", [128, e_pad // 16], dt.int16, kind="ExternalInput")
    dstidx_d = nc.dram_tensor("dstidx", [128, e_pad // 16], dt.int16, kind="ExternalInput")
    ind_d = nc.dram_tensor("ind", [128, nsub_total * 128], dt.float16, kind="ExternalInput")
    ident_d = nc.dram_tensor("ident", [128, 128], dt.float32, kind="ExternalInput")
    wts = []
    for li in range(NUM_LAYERS):
        dim = in_dim if li == 0 else HC
        wts.append(dict(
            wl=nc.dram_tensor(f"wl{li}", [dim, HC], dt.float32, kind="ExternalInput"),
            wr=nc.dram_tensor(f"wr{li}", [dim, HC], dt.float32, kind="ExternalInput"),
            att=nc.dram_tensor(f"att{li}", [128, HC], dt.float32, kind="ExternalInput"),
            blr=nc.dram_tensor(f"blr{li}", [128, HC], dt.float32, kind="ExternalInput"),
            brr=nc.dram_tensor(f"brr{li}", [128, HC], dt.float32, kind="ExternalInput"),
            bias=nc.dram_tensor(f"bias{li}", [128, HC], dt.float32, kind="ExternalInput"),
            g=nc.dram_tensor(f"g{li}", [128, HC], dt.float32, kind="ExternalInput"),
            b2=nc.dram_tensor(f"b2{li}", [128, HC], dt.float32, kind="ExternalInput"),
        ))
    y = nc.dram_tensor("y", [dpcp, HC], dt.float32, kind="ExternalOutput")

    with tile.TileContext(nc, num_cores=ncores) as tc:
        with (
            tc.tile_pool(name="const", bufs=1) as cpool,
            tc.tile_pool(name="idxp", bufs=1) as idxpool,
            tc.tile_pool(name="wt", bufs=1) as wtpool,
            tc.tile_pool(name="node", bufs=3) as npool,
            tc.tile_pool(name="edge", bufs=2) as epool,
            tc.tile_pool(name="msgp", bufs=2) as mpool,
            tc.tile_pool(name="small", bufs=4) as spool,
            tc.tile_pool(name="psA", bufs=2, space="PSUM") as psA,
            tc.tile_pool(name="psB", bufs=2, space="PSUM") as psB,
            tc.tile_pool(name="psC", bufs=2, space="PSUM") as psC,
            tc.tile_pool(name="dram", bufs=1, space="DRAM") as dpool,
        ):
            ident = cpool.tile([128, 128], dt.float32, tag="ident")
            nc.sync.dma_start(ident[:], ident_d[:])
            srcidx = idxpool.tile([128, e_pad // 16], dt.int16, tag="srcidx")
            nc.sync.dma_start(srcidx[:], srcidx_d[:])
            dstidx = idxpool.tile([128, e_pad // 16], dt.int16, tag="dstidx")
            nc.sync.dma_start(dstidx[:], dstidx_d[:])

            h_cur = h0  # dram handle of current layer input (f32)
            for li in range(n_layers):
                dim = in_dim if li == 0 else HC
                nj = dim // 128
                w = wts[li]

                # --- load weights / reps for this layer ---
                wl_s = wtpool.tile([128, nj, HC], dt.float32, tag="wl")
                wr_s = wtpool.tile([128, nj, HC], dt.float32, tag="wr")
                for j in range(nj):
                    nc.sync.dma_start(wl_s[:, j, :], w["wl"][j * 128:(j + 1) * 128, :])
                    nc.sync.dma_start(wr_s[:, j, :], w["wr"][j * 128:(j + 1) * 128, :])
                att_s = wtpool.tile([128, HC], dt.float32, tag="att")
                nc.sync.dma_start(att_s[:], w["att"][:])
                blr_s = wtpool.tile([128, HC], dt.float32, tag="blr")
                nc.sync.dma_start(blr_s[:], w["blr"][:])
                brr_s = wtpool.tile([128, HC], dt.float32, tag="brr")
                nc.sync.dma_start(brr_s[:], w["brr"][:])
                bias_s = wtpool.tile([128, HC], dt.float32, tag="bias")
                nc.sync.dma_start(bias_s[:], w["bias"][:])
                g_s = wtpool.tile([128, HC], dt.float32, tag="g")
                nc.sync.dma_start(g_s[:], w["g"][:])
                b2_s = wtpool.tile([128, HC], dt.float32, tag="b2")
                nc.sync.dma_start(b2_s[:], w["b2"][:])

                xlsh = dpool.tile([dpcp, HC], ED)
                xrloc = dpool.tile([dpcp, HC], ED)
                xlfull = dpool.tile([nfull, HC], ED)
                h_next = (
                    dpool.tile([dpcp, HC], dt.float32, name=f"h_next{li}")
                    if li < n_layers - 1
                    else None
                )

                # ---------------- node phase ----------------
                for nt in range([dpcp // 128, 0][skip_node]):
                    h_t = npool.tile([128, dim], dt.float32, tag="h_t")
                    nc.sync.dma_start(h_t[:], h_cur[nt * 128:(nt + 1) * 128, :])
                    hT = npool.tile([128, nj, 128], dt.float32, tag="hT")
                    for j in range(nj):
                        tp = psB.tile([128, 128], dt.float32, tag="tp")
                        nc.tensor.transpose(tp[:], h_t[:, j * 128:(j + 1) * 128], ident[:])
                        nc.scalar.activation(hT[:, j, :], tp[:], AF.Copy)
                    xl_ps = psA.tile([128, HC], dt.float32, tag="xl_ps")
                    for j in range(nj):
                        nc.tensor.matmul(xl_ps[:], lhsT=hT[:, j, :], rhs=wl_s[:, j, :],
                                         start=(j == 0), stop=(j == nj - 1))
                    xl16 = npool.tile([128, HC], ED, tag="xl16")
                    nc.vector.tensor_tensor(out=xl16[:], in0=xl_ps[:], in1=blr_s[:], op=OP.add)
                    nc.sync.dma_start(xlsh[nt * 128:(nt + 1) * 128, :], xl16[:])
                    xr_ps = psA.tile([128, HC], dt.float32, tag="xr_ps")
                    for j in range(nj):
                        nc.tensor.matmul(xr_ps[:], lhsT=hT[:, j, :], rhs=wr_s[:, j, :],
                                         start=(j == 0), stop=(j == nj - 1))
                    xr16 = npool.tile([128, HC], ED, tag="xr16")
                    nc.vector.tensor_tensor(out=xr16[:], in0=xr_ps[:], in1=brr_s[:], op=OP.add)
                    nc.sync.dma_start(xrloc[nt * 128:(nt + 1) * 128, :], xr16[:])

                if skip_node:
                    z16 = npool.tile([128, HC], ED, tag="z16")
                    nc.vector.memset(z16[:], 0)
                    for nt in range(dpcp // 128):
                        nc.sync.dma_start(xlsh[nt * 128:(nt + 1) * 128, :], z16[:])
                        nc.sync.dma_start(xrloc[nt * 128:(nt + 1) * 128, :], z16[:])
                # ---------------- allgather xl ----------------
                nc.gpsimd.collective_compute(
                    "AllGather", OP.bypass,
                    replica_groups=[list(range(ncores))],
                    ins=[xlsh.opt()], outs=[xlfull.opt()],
                )

                # ---------------- edge phase ----------------
                cum_sub = 0
                cum_src16 = 0
                cum_dst16 = 0
                for b in range(nb):
                    nsb = int(nsub_b[b])
                    if nsb == 0 or b >= max_blocks:
                        continue
                    Sb = nsb * 128
                    ind_t = epool.tile([128, nsb, 128], dt.float16, tag="ind_t")
                    if skip_ind:
                        nc.vector.memset(ind_t[:], 0)
                    if not skip_ind:
                        nc.sync.dma_start(
                            ind_t[:].rearrange("p a b -> p (a b)"),
                            ind_d[:, cum_sub * 128: (cum_sub + nsb) * 128],
                        )
                    # xr gather (whole block)
                    xr_g = epool.tile([128, nsb, HC], ED, tag="xr_g")
                    if skip_xrg:
                        nc.vector.memset(xr_g[:], 0)
                    if not skip_xrg:
                        for c0 in range(0, nsb, 8):
                            c1 = min(c0 + 8, nsb)
                            Sc = (c1 - c0) * 128
                            nc.gpsimd.dma_gather(
                                out_ap=xr_g[:, c0:c1, :],
                                in_ap=xrloc[:],
                                idxs_ap=dstidx[:, cum_dst16 + c0 * 8: cum_dst16 + c0 * 8 + Sc // 16],
                                num_idxs=Sc, num_idxs_reg=Sc, elem_size=HC,
                            )
                    cum_dst16 += Sb // 16
                    # xl gathers (per half) + add into xr_g
                    off_sub = 0
                    for h in (0, 1):
                        S = int(pad_sizes[b, h])
                        if S == 0:
                            continue
                        nsh = S // 128
                        xl_g = epool.tile([128, nsh, HC], ED, tag=f"xl_g{h}")
                        src_ap = xlfull[h * half:(h + 1) * half, :]
                        if skip_xlg:
                            nc.vector.memset(xl_g[:], 0)
                        if not skip_xlg:
                            for c0 in range(0, nsh, 8):
                                c1 = min(c0 + 8, nsh)
                                Sc = (c1 - c0) * 128
                                nc.gpsimd.dma_gather(
                                    out_ap=xl_g[:, c0:c1, :],
                                    in_ap=src_ap,
                                    idxs_ap=srcidx[:, cum_src16 + c0 * 8: cum_src16 + c0 * 8 + Sc // 16],
                                    num_idxs=Sc, num_idxs_reg=Sc, elem_size=HC,
                                )
                        cum_src16 += S // 16
                        # s = xl + xr  (into xr_g slice)
                        if edge_ops > 1:
                            nc.vector.tensor_tensor(
                                out=xr_g[:, off_sub:off_sub + nsh, :],
                                in0=xr_g[:, off_sub:off_sub + nsh, :],
                                in1=xl_g[:], op=OP.add,
                            )
                        # msg = xl * ex  -- deferred below (need ex); keep xl_g
                        # we instead compute msg later from xl_g tiles, so
                        # remember them:
                        if h == 0:
                            xl_g0, nsh0 = xl_g, nsh
                        else:
                            xl_g1, nsh1 = xl_g, nsh
                        off_sub += nsh
                    if edge_ops <= 1:
                        hn = npool.tile([128, HC], dt.float32, tag="hn")
                        nc.vector.tensor_copy(out=hn[:], in_=xr_g[:, 0, :])
                        dst_dram = y if li == n_layers - 1 else h_next
                        nc.sync.dma_start(dst_dram[b * 128:(b + 1) * 128, :], hn[:])
                        cum_sub += nsb
                        continue
                    # leaky (in place on s)
                    nc.vector.scalar_tensor_tensor(
                        out=xr_g[:], in0=xr_g[:], scalar=NEG_SLOPE, in1=xr_g[:],
                        op0=OP.mult, op1=OP.max,
                    )
                    # t = l * att (in place)
                    att_b = bass.AP(att_s[:].tensor, att_s[:].offset,
                                    [att_s[:].ap[0], [0, nsb], att_s[:].ap[1]])
                    nc.vector.tensor_tensor(out=xr_g[:], in0=xr_g[:], in1=att_b, op=OP.mult)
                    # logits: per-head sum over 64
                    logits = spool.tile([128, nsb, HEADS], dt.float32, tag="logits")
                    nc.vector.tensor_reduce(
                        out=logits[:],
                        in_=xr_g[:].rearrange("p a (h c) -> p a h c", c=HIDDEN),
                        axis=mybir.AxisListType.X, op=OP.add,
                    )
                    ex = spool.tile([128, nsb, HEADS], dt.float32, tag="ex")
                    nc.scalar.activation(
                        ex[:].rearrange("p a h -> p (a h)"),
                        logits[:].rearrange("p a h -> p (a h)"), AF.Exp)
                    if edge_ops <= 2:
                        hn = npool.tile([128, HC], dt.float32, tag="hn")
                        nc.vector.tensor_copy(out=hn[:], in_=t32[:, 0, :])
                        dst_dram = y if li == n_layers - 1 else h_next
                        nc.sync.dma_start(dst_dram[b * 128:(b + 1) * 128, :], hn[:])
                        cum_sub += nsb
                        continue
                    # msg = [xl * ex | ex]
                    msg = mpool.tile([128, nsb, HC + HEADS], ED, tag="msg")
                    off_sub = 0
                    for h in (0, 1):
                        S = int(pad_sizes[b, h])
                        if S == 0:
                            continue
                        xg = xl_g0 if h == 0 else xl_g1
                        nsh = S // 128
                        exs = ex[:, off_sub:off_sub + nsh, :]
                        ex_b = bass.AP(exs.tensor, exs.offset, exs.ap + [[0, HIDDEN]])
                        nc.vector.tensor_tensor(
                            out=msg[:, off_sub:off_sub + nsh, 0:HC].rearrange(
                                "p a (h c) -> p a h c", c=HIDDEN),
                            in0=xg[:].rearrange("p a (h c) -> p a h c", c=HIDDEN),
                            in1=ex_b, op=OP.mult,
                        )
                        off_sub += nsh
                    nc.vector.tensor_copy(out=msg[:, :, HC:HC + HEADS], in_=ex[:])
                    # seg-sum matmuls
                    ps = psC.tile([128, HC + HEADS], dt.float32, tag="ps")
                    for st in range(nsb):
                        nc.tensor.matmul(ps[:], lhsT=ind_t[:, st, :], rhs=msg[:, st, :],
                                         start=(st == 0), stop=(st == nsb - 1))
                    if edge_ops <= 3:
                        hn = npool.tile([128, HC], dt.float32, tag="hn")
                        nc.scalar.activation(hn[:], ps[:, 0:HC], AF.Copy)
                        dst_dram = y if li == n_layers - 1 else h_next
                        nc.sync.dma_start(dst_dram[b * 128:(b + 1) * 128, :], hn[:])
                        cum_sub += nsb
                        continue
                    # evacuate: out = ps[:, :256] / denom ; bias; relu; LN
                    den = spool.tile([128, HEADS], dt.float32, tag="den")
                    nc.vector.tensor_scalar_add(den[:], ps[:, HC:HC + HEADS], 1e-20)
                    rec = spool.tile([128, HEADS], dt.float32, tag="rec")
                    nc.vector.reciprocal(rec[:], den[:])
                    outv = npool.tile([128, HC], dt.float32, tag="outv")
                    rec_ap = rec[:]
                    rec_b = bass.AP(rec_ap.tensor, rec_ap.offset, rec_ap.ap + [[0, HIDDEN]])
                    nc.vector.tensor_tensor(
                        out=outv[:].rearrange("p (h c) -> p h c", c=HIDDEN),
                        in0=ps[:, 0:HC].rearrange("p (h c) -> p h c", c=HIDDEN),
                        in1=rec_b, op=OP.mult,
                    )
                    if edge_ops <= 4:
                        dst_dram = y if li == n_layers - 1 else h_next
                        nc.sync.dma_start(dst_dram[b * 128:(b + 1) * 128, :], outv[:])
                        cum_sub += nsb
                        continue
                    nc.vector.tensor_tensor(out=outv[:], in0=outv[:], in1=bias_s[:], op=OP.add)
                    nc.scalar.activation(outv[:], outv[:], AF.Relu)
                    # layernorm
                    if edge_ops <= 5:
                        dst_dram = y if li == n_layers - 1 else h_next
                        nc.sync.dma_start(dst_dram[b * 128:(b + 1) * 128, :], outv[:])
                        cum_sub += nsb
                        continue
                    negmu = spool.tile([128, 1], dt.float32, tag="negmu")
                    nc.vector.tensor_reduce(out=negmu[:], in_=outv[:],
                                            axis=mybir.AxisListType.X, op=OP.add,
                                            negate=True)
                    nc.scalar.mul(negmu[:], negmu[:], 1.0 / HC)
                    dcen = npool.tile([128, HC], dt.float32, tag="dcen")
                    nc.scalar.activation(dcen[:], outv[:], AF.Identity, bias=negmu[:, 0:1])
                    if edge_ops <= 6:
                        dst_dram = y if li == n_layers - 1 else h_next
                        nc.sync.dma_start(dst_dram[b * 128:(b + 1) * 128, :], dcen[:])
                        cum_sub += nsb
                        continue
                    junk = npool.tile([128, HC], dt.float32, tag="junk")
                    nc.vector.tensor_tensor(out=junk[:], in0=dcen[:], in1=dcen[:], op=OP.mult)
                    ssq = spool.tile([128, 1], dt.float32, tag="ssq")
                    nc.vector.tensor_reduce(out=ssq[:], in_=junk[:],
                                            axis=mybir.AxisListType.X, op=OP.add)
                    var = spool.tile([128, 1], dt.float32, tag="var")
                    nc.vector.tensor_scalar(out=var[:], in0=ssq[:], scalar1=1.0 / HC,
                                            scalar2=LN_EPS, op0=OP.mult, op1=OP.add)
                    sd = spool.tile([128, 1], dt.float32, tag="sd")
                    nc.scalar.activation(sd[:], var[:], AF.Sqrt)
                    rstd = spool.tile([128, 1], dt.float32, tag="rstd")
                    nc.vector.reciprocal(rstd[:], sd[:])
                    if edge_ops <= 7:
                        dst_dram = y if li == n_layers - 1 else h_next
                        nc.sync.dma_start(dst_dram[b * 128:(b + 1) * 128, :], dcen[:])
                        cum_sub += nsb
                        continue
                    hn = npool.tile([128, HC], dt.float32, tag="hn")
                    nc.vector.scalar_tensor_tensor(
                        out=hn[:], in0=dcen[:], scalar=rstd[:, 0:1], in1=g_s[:],
                        op0=OP.mult, op1=OP.mult,
                    )
                    nc.vector.tensor_tensor(out=hn[:], in0=hn[:], in1=b2_s[:], op=OP.add)
                    dst_dram = y if li == n_layers - 1 else h_next
                    nc.sync.dma_start(dst_dram[b * 128:(b + 1) * 128, :], hn[:])
                    cum_sub += nsb
                h_cur = h_next

    nc.finalize()
    return nc


def _make_inputs(x, params_np, per_core, cfg):
    dpc, dpcp, nfull, half, nb = _derive(cfg)
    n, in_dim = cfg["n"], cfg["in_dim"]
    ident = np.eye(128, dtype=np.float32)
    shared = {"ident": ident}
    for li, p in enumerate(params_np):
        rep = lambda v: np.tile(np.asarray(v, np.float32)[None, :], (128, 1))
        att_flat = np.asarray(p["att"], np.float32).reshape(-1)  # [h*64]
        shared[f"wl{li}"] = np.asarray(p["Wl"], np.float32)
        shared[f"wr{li}"] = np.asarray(p["Wr"], np.float32)
        shared[f"att{li}"] = np.tile(att_flat[None, :], (128, 1))
        shared[f"blr{li}"] = rep(p["bl"])
        shared[f"brr{li}"] = rep(p["br"])
        shared[f"bias{li}"] = rep(p["bias"])
        shared[f"g{li}"] = rep(p["ln_g"])
        shared[f"b2{li}"] = rep(p["ln_b"])
    in_maps = []
    for k in range(cfg["ncores"]):
        h0 = np.zeros((dpcp, in_dim), np.float32)
        h0[:dpc] = np.asarray(x[k * dpc:(k + 1) * dpc], np.float32)
        m = dict(shared)
        m["h0"] = h0
        m["srcidx"] = per_core[k]["srcidx"]
        m["dstidx"] = per_core[k]["dstidx"]
        m["ind"] = per_core[k]["ind"]
        in_maps.append(m)
    return in_maps


def run(x, edge_index, params, cfg=None, trace=False):
    cfg = cfg or _cfg_default()
    dpc, dpcp, nfull, half, nb = _derive(cfg)
    params_np = [{k: np.asarray(v) for k, v in p.items()} for p in params]
    per_core, meta = _preprocess(edge_index, cfg)
    nc = _build_program(params_np, meta, cfg)
    in_maps = _make_inputs(np.asarray(x), params_np, per_core, cfg)

    from concourse.bass_utils import run_bass_kernel_spmd
    res = run_bass_kernel_spmd(nc, in_maps, list(range(cfg["ncores"])), trace=trace)
    out = np.concatenate([res.results[k]["y"][:dpc] for k in range(cfg["ncores"])], axis=0)
    return out.astype(np.float32), res


def kernel(x, edge_index, params):
    try:
        import axon_prof  # noqa: F401  (enables NTFF tracing when present)
    except Exception:
        pass
    out, _ = run(x, edge_index, params)
    return out


# revision 14
# speedup vs baseline: 1.3398x; 1.3398x over previous
"""GATv2 backbone (3 layers, 4 heads x 64) on 8 Trainium2 NeuronCores.

Strategy (graph/data parallel):
  - dst-shard nodes: core k owns dst nodes [k*6250, (k+1)*6250), padded to 6272.
  - edges assigned to owner of dst, sorted by (dst-block of 128, src-half),
    padded per group to a size shared by all cores (SPMD: one program).
  - per layer: node-local xl/xr matmuls (fp16 out) -> AllGather xl ->
    edge phase: dma_gather rows of xl/xr per edge, DVE add+leaky+att-mul,
    per-head reduce -> logits -> exp (no max shift: logits in [-3, 3]) ->
    indicator-matrix matmul accumulates [msg | ex] into PSUM per dst block
    (segment softmax numerator + denominator in one pass) -> normalize,
    bias, relu, layernorm per block.
"""
import math
import numpy as np

# ---------------- problem constants (hardcoded per contract) ----------------
N_NODES = 50000
N_EDGES = 800000
IN_DIM = 128
HIDDEN = 64
HEADS = 4
HC = HEADS * HIDDEN  # 256
NUM_LAYERS = 3
NEG_SLOPE = 0.2
LN_EPS = 1e-5
NCORES = 8

_RUNNER = {}


def _cfg_default():
    return dict(n=N_NODES, e=N_EDGES, in_dim=IN_DIM, ncores=NCORES)


def _derive(cfg):
    n, ncores = cfg["n"], cfg["ncores"]
    dpc = n // ncores                      # dst nodes per core
    assert dpc * ncores == n
    dpcp = ((dpc + 127) // 128) * 128      # padded
    nfull = ncores * dpcp
    half = nfull // 2
    assert half % 128 == 0
    nb = dpcp // 128                       # dst blocks per core
    return dpc, dpcp, nfull, half, nb


def _wrap16(flat):
    """[S] int16 -> [128, S//16] wrapped-in-16-partitions, replicated x8."""
    assert flat.shape[0] % 16 == 0
    seg = flat.reshape(-1, 16).T  # [16, S/16]
    return np.tile(seg, (8, 1)).astype(np.int16)


def _preprocess(edge_index, cfg):
    """Build per-core edge structures with sizes shared across cores."""
    n, ncores = cfg["n"], cfg["ncores"]
    dpc, dpcp, nfull, half, nb = _derive(cfg)

    ei = np.asarray(edge_index).astype(np.int64)
    loop = np.arange(n, dtype=np.int64)
    src = np.concatenate([ei[0], loop])
    dst = np.concatenate([ei[1], loop])

    owner = dst // dpc
    dl = dst - owner * dpc                 # local dst 0..dpc-1
    blk = dl // 128
    gsrc = (src // dpc) * dpcp + (src % dpc)   # row in xl_full layout
    sh = (gsrc >= half).astype(np.int64)       # src half

    # group sizes per (core, block, half)
    key = (owner * nb + blk) * 2 + sh
    sizes = np.bincount(key, minlength=ncores * nb * 2).reshape(ncores, nb, 2)
    pad_sizes = sizes.max(axis=0)          # [nb, 2] shared across cores
    pad_sizes = ((pad_sizes + 127) // 128) * 128

    nsub_b = pad_sizes.sum(axis=1) // 128  # subtiles per block
    nsub_total = int(nsub_b.sum())
    e_pad = nsub_total * 128

    per_core = []
    for k in range(ncores):
        m = owner == k
        gs_k, dl_k, blk_k, sh_k = gsrc[m], dl[m], blk[m], sh[m]
        src_l = np.zeros(e_pad, np.int64)
        dst_l = np.zeros(e_pad, np.int64)
        real = np.zeros(e_pad, bool)
        pos = 0
        src_cols = []
        dst_cols = []
        for b in range(nb):
            bpos = pos
            for h in (0, 1):
                S = int(pad_sizes[b, h])
                if S == 0:
                    continue
                sel = (blk_k == b) & (sh_k == h)
                cnt = int(sel.sum())
                src_l[pos:pos + cnt] = gs_k[sel] - h * half
                dst_l[pos:pos + cnt] = dl_k[sel]
                real[pos:pos + cnt] = True
                # pad: src row 0 of this half, dst = b*128 (masked by Ind=0)
                dst_l[pos + cnt:pos + S] = b * 128
                src_cols.append(_wrap16(src_l[pos:pos + S].astype(np.int16)))
                pos += S
            Sb = pos - bpos
            if Sb:
                dst_cols.append(_wrap16(dst_l[bpos:pos].astype(np.int16)))
        assert pos == e_pad
        # indicator: [e_part 128, nsub, 128 dst] fp16
        st_of_edge = np.arange(e_pad) // 128
        blk_of_st = np.repeat(
            np.concatenate([np.full(int(nsub_b[b]), b) for b in range(nb)]), 128
        )
        d_in_blk = dst_l - blk_of_st * 128
        ind = np.zeros((e_pad, 128), np.float16)
        rr = np.nonzero(real)[0]
        ind[rr, d_in_blk[rr]] = 1.0
        ind3 = ind.reshape(nsub_total, 128, 128)
        ind_dev = np.ascontiguousarray(ind3.transpose(1, 0, 2).reshape(128, -1))
        indb_dev = np.ascontiguousarray(ind3.transpose(2, 0, 1).reshape(128, -1))
        per_core.append(
            dict(
                srcidx=np.ascontiguousarray(np.concatenate(src_cols, axis=1)),
                dstidx=np.ascontiguousarray(np.concatenate(dst_cols, axis=1)),
                ind=ind_dev,
                indb=indb_dev,
            )
        )
    meta = dict(pad_sizes=pad_sizes, nsub_b=nsub_b, nsub_total=nsub_total, e_pad=e_pad)
    return per_core, meta


def _build_program(params_np, meta, cfg):
    import concourse.bass as bass
    import concourse.bacc as bacc
    import concourse.mybir as mybir
    import concourse.tile as tile

    dt = mybir.dt
    AF = mybir.ActivationFunctionType
    OP = mybir.AluOpType

    n, ncores, in_dim = cfg["n"], cfg["ncores"], cfg["in_dim"]
    dpc, dpcp, nfull, half, nb = _derive(cfg)
    pad_sizes, nsub_b = meta["pad_sizes"], meta["nsub_b"]
    nsub_total, e_pad = meta["nsub_total"], meta["e_pad"]

    ED = dt.float16  # edge-phase dtype

    n_layers = cfg.get("layers", NUM_LAYERS)
    max_blocks = cfg.get("max_blocks", 10**9)
    edge_ops = cfg.get("edge_ops", 9)
    import os
    skip_xlg = bool(os.environ.get("SKIP_XLG"))
    skip_xrg = bool(os.environ.get("SKIP_XRG"))
    skip_ind = bool(os.environ.get("SKIP_IND"))
    skip_node = bool(os.environ.get("SKIP_NODE"))
    nc = bacc.Bacc("TRN2", target_bir_lowering=False, debug=False,
                   num_devices=ncores)

    h0 = nc.dram_tensor("h0", [dpcp, in_dim], dt.float32, kind="ExternalInput")
    srcidx_d = nc.dram_tensor("srcidx", [128, e_pad // 16], dt.int16, kind="ExternalInput")
    dstidx_d = nc.dram_tensor("dstidx", [128, e_pad // 16], dt.int16, kind="ExternalInput")
    ind_d = nc.dram_tensor("ind", [128, nsub_total * 128], dt.float16, kind="ExternalInput")
    indb_d = nc.dram_tensor("indb", [128, nsub_total * 128], dt.float16, kind="ExternalInput")
    ident_d = nc.dram_tensor("ident", [128, 128], dt.float32, kind="ExternalInput")
    wts = []
    for li in range(NUM_LAYERS):
        dim = in_dim if li == 0 else HC
        wts.append(dict(
            wl=nc.dram_tensor(f"wl{li}", [dim, HC], dt.float32, kind="ExternalInput"),
            wr=nc.dram_tensor(f"wr{li}", [dim, HC], dt.float32, kind="ExternalInput"),
            att=nc.dram_tensor(f"att{li}", [128, HC], dt.float32, kind="ExternalInput"),
            blr=nc.dram_tensor(f"blr{li}", [128, HC], dt.float32, kind="ExternalInput"),
            brr=nc.dram_tensor(f"brr{li}", [128, HC], dt.float32, kind="ExternalInput"),
            bias=nc.dram_tensor(f"bias{li}", [128, HC], dt.float32, kind="ExternalInput"),
            g=nc.dram_tensor(f"g{li}", [128, HC], dt.float32, kind="ExternalInput"),
            b2=nc.dram_tensor(f"b2{li}", [128, HC], dt.float32, kind="ExternalInput"),
        ))
    y = nc.dram_tensor("y", [dpcp, HC], dt.float32, kind="ExternalOutput")

    with tile.TileContext(nc, num_cores=ncores) as tc:
        with (
            tc.tile_pool(name="const", bufs=1) as cpool,
            tc.tile_pool(name="idxp", bufs=1) as idxpool,
            tc.tile_pool(name="wt", bufs=1) as wtpool,
            tc.tile_pool(name="node", bufs=3) as npool,
            tc.tile_pool(name="edge", bufs=2) as epool,
            tc.tile_pool(name="msgp", bufs=2) as mpool,
            tc.tile_pool(name="small", bufs=4) as spool,
            tc.tile_pool(name="psA", bufs=1, space="PSUM") as psA,
            tc.tile_pool(name="psB", bufs=2, space="PSUM") as psB,
            tc.tile_pool(name="psC", bufs=2, space="PSUM") as psC,
            tc.tile_pool(name="psD", bufs=2, space="PSUM") as psD,
            tc.tile_pool(name="dram", bufs=1, space="DRAM") as dpool,
        ):
            ident = cpool.tile([128, 128], dt.float32, tag="ident")
            nc.sync.dma_start(ident[:], ident_d[:])
            srcidx = idxpool.tile([128, e_pad // 16], dt.int16, tag="srcidx")
            nc.sync.dma_start(srcidx[:], srcidx_d[:])
            dstidx = idxpool.tile([128, e_pad // 16], dt.int16, tag="dstidx")
            nc.sync.dma_start(dstidx[:], dstidx_d[:])

            h_cur = h0  # dram handle of current layer input (f32)
            for li in range(n_layers):
                dim = in_dim if li == 0 else HC
                nj = dim // 128
                w = wts[li]

                # --- load weights / reps for this layer ---
                wl_s = wtpool.tile([128, nj, HC], dt.float32, tag="wl")
                wr_s = wtpool.tile([128, nj, HC], dt.float32, tag="wr")
                for j in range(nj):
                    nc.sync.dma_start(wl_s[:, j, :], w["wl"][j * 128:(j + 1) * 128, :])
                    nc.sync.dma_start(wr_s[:, j, :], w["wr"][j * 128:(j + 1) * 128, :])
                att_s = wtpool.tile([128, HC], dt.float32, tag="att")
                nc.sync.dma_start(att_s[:], w["att"][:])
                blr_s = wtpool.tile([128, HC], dt.float32, tag="blr")
                nc.sync.dma_start(blr_s[:], w["blr"][:])
                brr_s = wtpool.tile([128, HC], dt.float32, tag="brr")
                nc.sync.dma_start(brr_s[:], w["brr"][:])
                bias_s = wtpool.tile([128, HC], dt.float32, tag="bias")
                nc.sync.dma_start(bias_s[:], w["bias"][:])
                g_s = wtpool.tile([128, HC], dt.float32, tag="g")
                nc.sync.dma_start(g_s[:], w["g"][:])
                b2_s = wtpool.tile([128, HC], dt.float32, tag="b2")
                nc.sync.dma_start(b2_s[:], w["b2"][:])

                xlsh = dpool.tile([dpcp, HC], ED)
                xrloc = dpool.tile([dpcp, HC], ED)
                xlfull = dpool.tile([nfull, HC], ED)
                h_next = (
                    dpool.tile([dpcp, HC], dt.float32, name=f"h_next{li}")
                    if li < n_layers - 1
                    else None
                )

                # ---------------- node phase ----------------
                for nt in range([dpcp // 128, 0][skip_node]):
                    h_t = npool.tile([128, dim], dt.float32, tag="h_t")
                    nc.sync.dma_start(h_t[:], h_cur[nt * 128:(nt + 1) * 128, :])
                    hT = npool.tile([128, nj, 128], dt.float32, tag="hT")
                    for j in range(nj):
                        tp = psB.tile([128, 128], dt.float32, tag="tp")
                        nc.tensor.transpose(tp[:], h_t[:, j * 128:(j + 1) * 128], ident[:])
                        nc.scalar.activation(hT[:, j, :], tp[:], AF.Copy)
                    xl_ps = psA.tile([128, HC], dt.float32, tag="xl_ps")
                    for j in range(nj):
                        nc.tensor.matmul(xl_ps[:], lhsT=hT[:, j, :], rhs=wl_s[:, j, :],
                                         start=(j == 0), stop=(j == nj - 1))
                    xl16 = npool.tile([128, HC], ED, tag="xl16")
                    nc.vector.tensor_tensor(out=xl16[:], in0=xl_ps[:], in1=blr_s[:], op=OP.add)
                    nc.sync.dma_start(xlsh[nt * 128:(nt + 1) * 128, :], xl16[:])
                    xr_ps = psA.tile([128, HC], dt.float32, tag="xr_ps")
                    for j in range(nj):
                        nc.tensor.matmul(xr_ps[:], lhsT=hT[:, j, :], rhs=wr_s[:, j, :],
                                         start=(j == 0), stop=(j == nj - 1))
                    xr16 = npool.tile([128, HC], ED, tag="xr16")
                    nc.vector.tensor_tensor(out=xr16[:], in0=xr_ps[:], in1=brr_s[:], op=OP.add)
                    nc.sync.dma_start(xrloc[nt * 128:(nt + 1) * 128, :], xr16[:])

                if skip_node:
                    z16 = npool.tile([128, HC], ED, tag="z16")
                    nc.vector.memset(z16[:], 0)
                    for nt in range(dpcp // 128):
                        nc.sync.dma_start(xlsh[nt * 128:(nt + 1) * 128, :], z16[:])
                        nc.sync.dma_start(xrloc[nt * 128:(nt + 1) * 128, :], z16[:])
                # ---------------- allgather xl ----------------
                nc.gpsimd.collective_compute(
                    "AllGather", OP.bypass,
                    replica_groups=[list(range(ncores))],
                    ins=[xlsh.opt()], outs=[xlfull.opt()],
                )

                # ---------------- edge phase ----------------
                cum_sub = 0
                cum_src16 = 0
                cum_dst16 = 0
                for b in range(nb):
                    nsb = int(nsub_b[b])
                    if nsb == 0 or b >= max_blocks:
                        continue
                    Sb = nsb * 128
                    ind_t = epool.tile([128, nsb, 128], dt.float16, tag="ind_t")
                    if skip_ind:
                        nc.vector.memset(ind_t[:], 0)
                    if not skip_ind:
                        nc.sync.dma_start(
                            ind_t[:].rearrange("p a b -> p (a b)"),
                            ind_d[:, cum_sub * 128: (cum_sub + nsb) * 128],
                        )
                    # xr gather (whole block)
                    xr_g = epool.tile([128, nsb, HC], ED, tag="xr_g")
                    indb_t = epool.tile([128, nsb, 128], dt.float16, tag="indb_t")
                    nc.sync.dma_start(
                        indb_t[:].rearrange("p a b -> p (a b)"),
                        indb_d[:, cum_sub * 128: (cum_sub + nsb) * 128],
                    )
                    xr_blk = spool.tile([128, HC], ED, tag="xr_blk")
                    nc.sync.dma_start(xr_blk[:], xrloc[b * 128:(b + 1) * 128, :])
                    for st in range(nsb):
                        ps_x = psD.tile([128, HC], dt.float32, tag="ps_x")
                        nc.tensor.matmul(ps_x[:], lhsT=indb_t[:, st, :], rhs=xr_blk[:],
                                         start=True, stop=True)
                        nc.scalar.activation(xr_g[:, st, :], ps_x[:], AF.Copy)
                    cum_dst16 += Sb // 16
                    # xl gathers (per half) + add into xr_g
                    off_sub = 0
                    for h in (0, 1):
                        S = int(pad_sizes[b, h])
                        if S == 0:
                            continue
                        nsh = S // 128
                        xl_g = epool.tile([128, nsh, HC], ED, tag=f"xl_g{h}")
                        src_ap = xlfull[h * half:(h + 1) * half, :]
                        if skip_xlg:
                            nc.vector.memset(xl_g[:], 0)
                        if not skip_xlg:
                            for c0 in range(0, nsh, 8):
                                c1 = min(c0 + 8, nsh)
                                Sc = (c1 - c0) * 128
                                nc.gpsimd.dma_gather(
                                    out_ap=xl_g[:, c0:c1, :],
                                    in_ap=src_ap,
                                    idxs_ap=srcidx[:, cum_src16 + c0 * 8: cum_src16 + c0 * 8 + Sc // 16],
                                    num_idxs=Sc, num_idxs_reg=Sc, elem_size=HC,
                                )
                        cum_src16 += S // 16
                        # s = xl + xr  (into xr_g slice)
                        if edge_ops > 1:
                            nc.vector.tensor_tensor(
                                out=xr_g[:, off_sub:off_sub + nsh, :],
                                in0=xr_g[:, off_sub:off_sub + nsh, :],
                                in1=xl_g[:], op=OP.add,
                            )
                        # msg = xl * ex  -- deferred below (need ex); keep xl_g
                        # we instead compute msg later from xl_g tiles, so
                        # remember them:
                        if h == 0:
                            xl_g0, nsh0 = xl_g, nsh
                        else:
                            xl_g1, nsh1 = xl_g, nsh
                        off_sub += nsh
                    if edge_ops <= 1:
                        hn = npool.tile([128, HC], dt.float32, tag="hn")
                        nc.vector.tensor_copy(out=hn[:], in_=xr_g[:, 0, :])
                        dst_dram = y if li == n_layers - 1 else h_next
                        nc.sync.dma_start(dst_dram[b * 128:(b + 1) * 128, :], hn[:])
                        cum_sub += nsb
                        continue
                    # leaky (in place on s)
                    nc.vector.scalar_tensor_tensor(
                        out=xr_g[:], in0=xr_g[:], scalar=NEG_SLOPE, in1=xr_g[:],
                        op0=OP.mult, op1=OP.max,
                    )
                    # t = l * att (in place)
                    att_b = bass.AP(att_s[:].tensor, att_s[:].offset,
                                    [att_s[:].ap[0], [0, nsb], att_s[:].ap[1]])
                    nc.vector.tensor_tensor(out=xr_g[:], in0=xr_g[:], in1=att_b, op=OP.mult)
                    # logits: per-head sum over 64
                    logits = spool.tile([128, nsb, HEADS], dt.float32, tag="logits")
                    nc.vector.tensor_reduce(
                        out=logits[:],
                        in_=xr_g[:].rearrange("p a (h c) -> p a h c", c=HIDDEN),
                        axis=mybir.AxisListType.X, op=OP.add,
                    )
                    ex = spool.tile([128, nsb, HEADS], dt.float32, tag="ex")
                    nc.scalar.activation(
                        ex[:].rearrange("p a h -> p (a h)"),
                        logits[:].rearrange("p a h -> p (a h)"), AF.Exp)
                    if edge_ops <= 2:
                        hn = npool.tile([128, HC], dt.float32, tag="hn")
                        nc.vector.tensor_copy(out=hn[:], in_=t32[:, 0, :])
                        dst_dram = y if li == n_layers - 1 else h_next
                        nc.sync.dma_start(dst_dram[b * 128:(b + 1) * 128, :], hn[:])
                        cum_sub += nsb
                        continue
                    # msg = [xl * ex | ex]
                    msg = mpool.tile([128, nsb, HC + HEADS], ED, tag="msg")
                    off_sub = 0
                    for h in (0, 1):
                        S = int(pad_sizes[b, h])
                        if S == 0:
                            continue
                        xg = xl_g0 if h == 0 else xl_g1
                        nsh = S // 128
                        exs = ex[:, off_sub:off_sub + nsh, :]
                        ex_b = bass.AP(exs.tensor, exs.offset, exs.ap + [[0, HIDDEN]])
                        nc.vector.tensor_tensor(
                            out=msg[:, off_sub:off_sub + nsh, 0:HC].rearrange(
                                "p a (h c) -> p a h c", c=HIDDEN),
                            in0=xg[:].rearrange("p a (h c) -> p a h c", c=HIDDEN),
                            in1=ex_b, op=OP.mult,
                        )
                        off_sub += nsh
                    nc.vector.tensor_copy(out=msg[:, :, HC:HC + HEADS], in_=ex[:])
                    # seg-sum matmuls
                    ps = psC.tile([128, HC + HEADS], dt.float32, tag="ps")
                    for st in range(nsb):
                        nc.tensor.matmul(ps[:], lhsT=ind_t[:, st, :], rhs=msg[:, st, :],
                                         start=(st == 0), stop=(st == nsb - 1))
                    if edge_ops <= 3:
                        hn = npool.tile([128, HC], dt.float32, tag="hn")
                        nc.scalar.activation(hn[:], ps[:, 0:HC], AF.Copy)
                        dst_dram = y if li == n_layers - 1 else h_next
                        nc.sync.dma_start(dst_dram[b * 128:(b + 1) * 128, :], hn[:])
                        cum_sub += nsb
                        continue
                    # evacuate: out = ps[:, :256] / denom ; bias; relu; LN
                    den = spool.tile([128, HEADS], dt.float32, tag="den")
                    nc.vector.tensor_scalar_add(den[:], ps[:, HC:HC + HEADS], 1e-20)
                    rec = spool.tile([128, HEADS], dt.float32, tag="rec")
                    nc.vector.reciprocal(rec[:], den[:])
                    outv = npool.tile([128, HC], dt.float32, tag="outv")
                    rec_ap = rec[:]
                    rec_b = bass.AP(rec_ap.tensor, rec_ap.offset, rec_ap.ap + [[0, HIDDEN]])
                    nc.vector.tensor_tensor(
                        out=outv[:].rearrange("p (h c) -> p h c", c=HIDDEN),
                        in0=ps[:, 0:HC].rearrange("p (h c) -> p h c", c=HIDDEN),
                        in1=rec_b, op=OP.mult,
                    )
                    if edge_ops <= 4:
                        dst_dram = y if li == n_layers - 1 else h_next
                        nc.sync.dma_start(dst_dram[b * 128:(b + 1) * 128, :], outv[:])
                        cum_sub += nsb
                        continue
                    nc.vector.tensor_tensor(out=outv[:], in0=outv[:], in1=bias_s[:], op=OP.add)
                    nc.scalar.activation(outv[:], outv[:], AF.Relu)
                    # layernorm
                    if edge_ops <= 5:
                        dst_dram = y if li == n_layers - 1 else h_next
                        nc.sync.dma_start(dst_dram[b * 128:(b + 1) * 128, :], outv[:])
                        cum_sub += nsb
                        continue
                    negmu = spool.tile([128, 1], dt.float32, tag="negmu")
                    nc.vector.tensor_reduce(out=negmu[:], in_=outv[:],
                                            axis=mybir.AxisListType.X, op=OP.add,
                                            negate=True)
                    nc.scalar.mul(negmu[:], negmu[:], 1.0 / HC)
                    dcen = npool.tile([128, HC], dt.float32, tag="dcen")
                    nc.scalar.activation(dcen[:], outv[:], AF.Identity, bias=negmu[:, 0:1])
                    if edge_ops <= 6:
                        dst_dram = y if li == n_layers - 1 else h_next
                        nc.sync.dma_start(dst_dram[b * 128:(b + 1) * 128, :], dcen[:])
                        cum_sub += nsb
                        continue
                    junk = npool.tile([128, HC], dt.float32, tag="junk")
                    nc.vector.tensor_tensor(out=junk[:], in0=dcen[:], in1=dcen[:], op=OP.mult)
                    ssq = spool.tile([128, 1], dt.float32, tag="ssq")
                    nc.vector.tensor_reduce(out=ssq[:], in_=junk[:],
                                            axis=mybir.AxisListType.X, op=OP.add)
                    var = spool.tile([128, 1], dt.float32, tag="var")
                    nc.vector.tensor_scalar(out=var[:], in0=ssq[:], scalar1=1.0 / HC,
                                            scalar2=LN_EPS, op0=OP.mult, op1=OP.add)
                    sd = spool.tile([128, 1], dt.float32, tag="sd")
                    nc.scalar.activation(sd[:], var[:], AF.Sqrt)
                    rstd = spool.tile([128, 1], dt.float32, tag="rstd")
                    nc.vector.reciprocal(rstd[:], sd[:])
                    if edge_ops <= 7:
                        dst_dram = y if li == n_layers - 1 else h_next
                        nc.sync.dma_start(dst_dram[b * 128:(b + 1) * 128, :], dcen[:])
                        cum_sub += nsb
                        continue
                    hn = npool.tile([128, HC], dt.float32, tag="hn")
                    nc.vector.scalar_tensor_tensor(
                        out=hn[:], in0=dcen[:], scalar=rstd[:, 0:1], in1=g_s[:],
                        op0=OP.mult, op1=OP.mult,
                    )
                    nc.vector.tensor_tensor(out=hn[:], in0=hn[:], in1=b2_s[:], op=OP.add)
                    dst_dram = y if li == n_layers - 1 else h_next
                    nc.sync.dma_start(dst_dram[b * 128:(b + 1) * 128, :], hn[:])
                    cum_sub += nsb
                h_cur = h_next

    nc.finalize()
    return nc


def _make_inputs(x, params_np, per_core, cfg):
    dpc, dpcp, nfull, half, nb = _derive(cfg)
    n, in_dim = cfg["n"], cfg["in_dim"]
    ident = np.eye(128, dtype=np.float32)
    shared = {"ident": ident}
    for li, p in enumerate(params_np):
        rep = lambda v: np.tile(np.asarray(v, np.float32)[None, :], (128, 1))
        att_flat = np.asarray(p["att"], np.float32).reshape(-1)  # [h*64]
        shared[f"wl{li}"] = np.asarray(p["Wl"], np.float32)
        shared[f"wr{li}"] = np.asarray(p["Wr"], np.float32)
        shared[f"att{li}"] = np.tile(att_flat[None, :], (128, 1))
        shared[f"blr{li}"] = rep(p["bl"])
        shared[f"brr{li}"] = rep(p["br"])
        shared[f"bias{li}"] = rep(p["bias"])
        shared[f"g{li}"] = rep(p["ln_g"])
        shared[f"b2{li}"] = rep(p["ln_b"])
    in_maps = []
    for k in range(cfg["ncores"]):
        h0 = np.zeros((dpcp, in_dim), np.float32)
        h0[:dpc] = np.asarray(x[k * dpc:(k + 1) * dpc], np.float32)
        m = dict(shared)
        m["h0"] = h0
        m["srcidx"] = per_core[k]["srcidx"]
        m["dstidx"] = per_core[k]["dstidx"]
        m["ind"] = per_core[k]["ind"]
        m["indb"] = per_core[k]["indb"]
        in_maps.append(m)
    return in_maps


def run(x, edge_index, params, cfg=None, trace=False):
    cfg = cfg or _cfg_default()
    dpc, dpcp, nfull, half, nb = _derive(cfg)
    params_np = [{k: np.asarray(v) for k, v in p.items()} for p in params]
    per_core, meta = _preprocess(edge_index, cfg)
    nc = _build_program(params_np, meta, cfg)
    in_maps = _make_inputs(np.asarray(x), params_np, per_core, cfg)

    from concourse.bass_utils import run_bass_kernel_spmd
    res = run_bass_kernel_spmd(nc, in_maps, list(range(cfg["ncores"])), trace=trace)
    out = np.concatenate([res.results[k]["y"][:dpc] for k in range(cfg["ncores"])], axis=0)
    return out.astype(np.float32), res


def kernel(x, edge_index, params):
    try:
        import axon_prof  # noqa: F401  (enables NTFF tracing when present)
    except Exception:
        pass
    out, _ = run(x, edge_index, params)
    return out
